# revision 18
# baseline (speedup 1.0000x reference)
"""GQA attention (B=4,S=1024,D=2048,H=32,KVH=8,HD=64) + RoPE, tensor-parallel
over the 8 kv-head groups across 8 NeuronCores.

Per-core pipeline (all-transposed layouts, no on-device softmax-max pass):
  qT/kT/vT = W.T @ xT         (PE, bf16, D-contraction in 16 chunks of 128)
  RoPE: qrot = (q*cos) + A@(q*sin)   (rotate-half folded into a PE matmul A)
  sT = k_ropeT.T-block @ q_ropeT     ([keys,q] scores, K=64 contraction)
  pT = exp(sT/8)                     (ACT, no max subtraction; scores are small)
  avT_aug = [v|1].T @ pT             (PE; row 64 = softmax denominator)
  avT = avT_aug[:64] * bcast(1/denom)
  y_partial = avT.T-block @ Wo_g     (PE), DMA out fp32; host sums the 8 partials.
"""

import numpy as np
import ml_dtypes

import concourse.bass as bass
import concourse.mybir as mybir
import concourse.tile as tile
from concourse import bacc
from concourse import bass_utils

BF16 = mybir.dt.bfloat16
F32 = mybir.dt.float32
BF = ml_dtypes.bfloat16

B, S, D = 4, 1024, 2048
H, KVH, HD = 32, 8, 64
NREP = H // KVH          # 4 q heads per core
T = B * S                # 4096 tokens
NC = 8                   # cores
QD = NREP * HD           # 256 q dims per core
KC = D // 128            # 16 contraction chunks
TB = 512                 # proj token-block
NTB = T // TB            # 8
AF = mybir.ActivationFunctionType

_CACHE = {}


def _build(debug=False):
    key = ("nc", debug)
    if key in _CACHE:
        return _CACHE[key]
    nc = bacc.Bacc("TRN2", target_bir_lowering=False)
    # Pin all ACT table lookups to set 6 (natural_log_exp_and_others: has
    # Exp, Ln, Copy) so the kernel needs exactly one table load instead of
    # thrashing between the exp-only and ln-only sets every head.
    import concourse.bacc as _bacc_mod
    _orig_tables = _bacc_mod.get_activation_tables

    def _pinned_tables(arch):
        items = list(_orig_tables(arch).items())
        return {k: (v if i == 6 else set()) for i, (k, v) in enumerate(items)}

    _bacc_mod.get_activation_tables = _pinned_tables
    dbg = {}
    if debug:
        for nm, shape, dt in [
            ("dump_q", (128, T), BF16), ("dump_k", (128, T), BF16),
            ("dump_vaug", (128, 8 * 65), BF16), ("dump_prob", (128, 8 * S), BF16),
            ("dump_ava", (65, S), F32), ("dump_rbc", (64, S), F32),
            ("dump_avn", (64, S), BF16),
        ]:
            dbg[nm] = nc.dram_tensor(nm, shape, dt, kind="ExternalOutput")

    xT_d = nc.dram_tensor("xT", (D, T), BF16, kind="ExternalInput")
    wq_d = nc.dram_tensor("wq", (D, QD), BF16, kind="ExternalInput")
    wkv_d = nc.dram_tensor("wkv", (D, 128), BF16, kind="ExternalInput")
    wo_d = nc.dram_tensor("wo", (QD, D), BF16, kind="ExternalInput")
    cos_d = nc.dram_tensor("cos2", (128, S), F32, kind="ExternalInput")
    sin_d = nc.dram_tensor("sin2", (128, S), F32, kind="ExternalInput")
    arot_d = nc.dram_tensor("arot", (128, 128), BF16, kind="ExternalInput")
    eye_d = nc.dram_tensor("eye64", (64, 64), BF16, kind="ExternalInput")
    y_d = nc.dram_tensor("y", (T, D), F32, kind="ExternalOutput")

    with tile.TileContext(nc) as tc:
        with (
            tc.tile_pool(name="const", bufs=1) as cpool,
            tc.tile_pool(name="persist", bufs=1) as ppool,
        ):
            # ---- constants ----
            # split the Wq load so the first K-chunks' matmuls can start
            # before the whole weight set has landed
            wq_sb = cpool.tile([128, KC * QD], BF16, tag="wq")
            wq_dv = wq_d[:].rearrange("(c p) m -> p c m", p=128)
            wq_sv = wq_sb[:].rearrange("p (c m) -> p c m", c=KC)
            nc.sync.dma_start(out=wq_sv[:, 0:4, :], in_=wq_dv[:, 0:4, :])
            wkv_sb = cpool.tile([128, KC * 128], BF16, tag="wkv")
            nc.sync.dma_start(
                out=wkv_sb[:].rearrange("p (c m) -> p c m", c=KC),
                in_=wkv_d[:].rearrange("(c p) m -> p c m", p=128),
            )
            nc.sync.dma_start(out=wq_sv[:, 4:KC, :], in_=wq_dv[:, 4:KC, :])
            # Wo as rhs chunks: head h rows -> [64, D] at partitions 0:64
            wo_sb = cpool.tile([64, NREP * D], BF16, tag="wo")
            nc.sync.dma_start(
                out=wo_sb[:].rearrange("p (h n) -> p h n", h=NREP),
                in_=wo_d[:].rearrange("(h p) n -> p h n", p=64),
            )
            cos_sb = cpool.tile([128, S], F32, tag="cos")
            nc.sync.dma_start(out=cos_sb[:], in_=cos_d[:])
            sin_sb = cpool.tile([128, S], F32, tag="sin")
            nc.sync.dma_start(out=sin_sb[:], in_=sin_d[:])
            arot_sb = cpool.tile([128, 128], BF16, tag="arot")
            nc.sync.dma_start(out=arot_sb[:], in_=arot_d[:])
            eye_sb = cpool.tile([64, 64], BF16, tag="eye")
            nc.sync.dma_start(out=eye_sb[:], in_=eye_d[:])

            # ---- persistent activations ----
            qrope = [ppool.tile([128, T], BF16, tag=f"qrope{p}", name=f"qrope{p}") for p in range(2)]
            kT2 = ppool.tile([128, T], BF16, tag="kT2")
            vT_sb = ppool.tile([128, T], BF16, tag="vT")      # rows 64:128 used
            vtmpT = ppool.tile([64, T], BF16, tag="vtmpT")
            v_aug = [ppool.tile([128, 8 * 65], BF16, tag=f"vaug{b}", name=f"vaug{b}") for b in range(B)]
            avn = [[None] * NREP for _ in range(B)]

            # ================= projection phase =================
            with (
                tc.tile_pool(name="xin", bufs=2) as xpool,
                tc.tile_pool(name="rtmp", bufs=3) as rpool,
                tc.tile_pool(name="pj", bufs=1, space="PSUM") as pj,
                tc.tile_pool(name="pshift", bufs=2, space="PSUM") as psh,
                tc.tile_pool(name="pvtr", bufs=2, space="PSUM") as pvt,
            ):
                for tb in range(NTB):
                    b, scol = tb // 2, (tb % 2) * TB
                    tcols = bass.ts(tb, TB)
                    xts = xpool.tile([128, KC * TB], BF16, tag="xts")
                    nc.sync.dma_start(
                        out=xts[:].rearrange("p (c n) -> p c n", c=KC),
                        in_=xT_d[:, tcols].rearrange("(c p) n -> p c n", p=128),
                    )
                    q0ps = pj.tile([128, TB], F32, tag="q0")
                    q1ps = pj.tile([128, TB], F32, tag="q1")
                    kvps = pj.tile([128, TB], F32, tag="kv")
                    for c in range(KC):
                        xc = xts[:, bass.ts(c, TB)]
                        st = dict(start=(c == 0), stop=(c == KC - 1))
                        nc.tensor.matmul(q0ps[:], wq_sb[:, c * QD:c * QD + 128], xc, **st)
                        nc.tensor.matmul(q1ps[:], wq_sb[:, c * QD + 128:(c + 1) * QD], xc, **st)
                        nc.tensor.matmul(kvps[:], wkv_sb[:, bass.ts(c, 128)], xc, **st)
                    css, sns = cos_sb[:, scol:scol + TB], sin_sb[:, scol:scol + TB]
                    # q pairs RoPE
                    for p, qps in ((0, q0ps), (1, q1ps)):
                        qsin = rpool.tile([128, TB], BF16, tag="qsin")
                        nc.vector.tensor_mul(qsin[:], qps[:], sns)
                        t1 = rpool.tile([128, TB], F32, tag="t1")
                        nc.vector.tensor_mul(t1[:], qps[:], css)
                        shift = psh.tile([128, TB], F32, tag="shift")
                        nc.tensor.matmul(shift[:], arot_sb[:], qsin[:], start=True, stop=True)
                        nc.vector.tensor_add(qrope[p][:, tcols], t1[:], shift[:])
                    # k RoPE on rows 0:64
                    ksin = rpool.tile([64, TB], BF16, tag="qsin")
                    nc.vector.tensor_mul(ksin[:], kvps[0:64, :], sns[0:64])
                    t1k = rpool.tile([64, TB], F32, tag="t1")
                    nc.vector.tensor_mul(t1k[:], kvps[0:64, :], css[0:64])
                    shk = psh.tile([128, TB], F32, tag="shift")
                    nc.tensor.matmul(shk[0:64, :], arot_sb[0:64, 0:64], ksin[:], start=True, stop=True)
                    nc.vector.tensor_add(kT2[0:64, tcols], t1k[:], shk[0:64, :])
                    # v: copy to rows 64:128, then DMA down to partitions 0:64
                    nc.scalar.copy(vT_sb[64:128, tcols], kvps[64:128, :])
                    nc.sync.dma_start(out=vtmpT[:, tcols], in_=vT_sb[64:128, tcols])
                    if tb % 2 == 1:
                        # batch b complete: build v natural (+ones col)
                        for kb in range(8):
                            vtr = pvt.tile([128, 64], BF16, tag="vtr")
                            nc.tensor.transpose(
                                vtr[:], vtmpT[:, b * S + kb * 128:b * S + (kb + 1) * 128],
                                eye_sb[:],
                            )
                            nc.scalar.copy(v_aug[b][:, kb * 65:kb * 65 + 64], vtr[:])
                        nc.vector.memset(
                            v_aug[b][:].rearrange("p (k o) -> p k o", k=8)[:, :, 64:65], 1.0
                        )
                # duplicate k_rope to rows 64:128 (for head-odd alignment)
                nc.sync.dma_start(out=kT2[64:128, :], in_=kT2[0:64, :])
                if debug:
                    nc.sync.dma_start(out=dbg["dump_q"][:], in_=qrope[0][:])
                    nc.sync.dma_start(out=dbg["dump_k"][:], in_=kT2[:])
                    nc.sync.dma_start(out=dbg["dump_vaug"][:], in_=v_aug[0][:])

            # ================= attention + output phase =================
            with (
                tc.tile_pool(name="prob", bufs=2) as prpool,
                tc.tile_pool(name="nrm", bufs=2) as npool,
                tc.tile_pool(name="avns", bufs=2 * NREP) as apool,
                tc.tile_pool(name="yout", bufs=2) as ypool,
                tc.tile_pool(name="psT", bufs=1, space="PSUM") as pst,
                tc.tile_pool(name="pav", bufs=1, space="PSUM") as pav,
                tc.tile_pool(name="py", bufs=2, space="PSUM") as py,
                tc.tile_pool(name="dscr", bufs=2, space="DRAM") as dpool,
            ):
                ycopy = 0
                for b in range(B):
                    for pr in range(2):        # head pair: heads 2pr, 2pr+1
                        # scores for both heads issued adjacently: they use
                        # disjoint partition halves (0:64 / 64:128), so the
                        # PE runs them concurrently in separate row-groups.
                        prob0 = prpool.tile([128, 8 * S], BF16, tag="prob", name="prob0")
                        prob1 = prpool.tile([128, 8 * S], BF16, tag="prob", name="prob1")
                        probs = (prob0, prob1)
                        avps0 = pav.tile([128, S], F32, tag="avT", name="avps0")
                        for kb in range(8):
                            sps0 = pst.tile([128, S], F32, tag="sT0", name="sps0")
                            sps1 = pst.tile([128, S], F32, tag="sT1", name="sps1")
                            spss = (sps0, sps1)
                            for qh in range(2):
                                for hh in range(2):
                                    r0 = hh * 64
                                    nc.tensor.matmul(
                                        spss[hh][:, bass.ts(qh, 512)],
                                        kT2[r0:r0 + 64, b * S + kb * 128:b * S + (kb + 1) * 128],
                                        qrope[pr][r0:r0 + 64, b * S + qh * 512:b * S + (qh + 1) * 512],
                                        start=True, stop=True,
                                    )
                            for hh in range(2):
                                nc.scalar.activation(
                                    probs[hh][:, bass.ts(kb, S)], spss[hh][:],
                                    AF.Exp, scale=0.125,
                                )
                            # AV for the even head rides along per-kb
                            for qh in range(2):
                                nc.tensor.matmul(
                                    avps0[0:65, bass.ts(qh, 512)],
                                    v_aug[b][:, kb * 65:(kb + 1) * 65],
                                    prob0[:, kb * S + qh * 512:kb * S + (qh + 1) * 512],
                                    start=(kb == 0), stop=(kb == 7),
                                )
                        for hh in range(2):
                            h = 2 * pr + hh
                            if hh == 0:
                                avps = avps0
                            else:
                                avps = pav.tile([128, S], F32, tag="avT", name="avps1")
                                for kb in range(8):
                                    for qh in range(2):
                                        nc.tensor.matmul(
                                            avps[0:65, bass.ts(qh, 512)],
                                            v_aug[b][:, kb * 65:(kb + 1) * 65],
                                            prob1[:, kb * S + qh * 512:kb * S + (qh + 1) * 512],
                                            start=(kb == 0), stop=(kb == 7),
                                        )
                            # 1/sums via exp(-ln(x)) on ACT (both fns share one
                            # table set); broadcast across partitions via DRAM.
                            lnr = npool.tile([65, S], F32, tag="lnr")
                            nc.scalar.activation(lnr[64:65, :], avps[64:65, :], AF.Ln)
                            rrow = npool.tile([65, S], F32, tag="rrow")
                            nc.scalar.activation(rrow[64:65, :], lnr[64:65, :], AF.Exp, scale=-1.0)
                            sdr = dpool.tile([1, S], F32, tag="sdr")
                            nc.sync.dma_start(out=sdr[:], in_=rrow[64:65, :])
                            rbc = npool.tile([64, S], F32, tag="rbc")
                            nc.gpsimd.dma_start(out=rbc[:], in_=sdr[:].to_broadcast((64, S)))
                            avn_t = apool.tile([64, S], BF16, tag="avn")
                            nc.vector.tensor_mul(avn_t[:], avps[0:64, :], rbc[:])
                            avn[b][h] = avn_t
                            if debug and b == 0 and h == 0:
                                nc.sync.dma_start(out=dbg["dump_prob"][:], in_=prob0[:])
                                ava_sb = npool.tile([65, S], F32, tag="avadbg")
                                nc.vector.tensor_copy(ava_sb[:], avps[0:65, :])
                                nc.sync.dma_start(out=dbg["dump_ava"][:], in_=ava_sb[:])
                                nc.sync.dma_start(out=dbg["dump_rbc"][:], in_=rbc[:])
                                nc.sync.dma_start(out=dbg["dump_avn"][:], in_=avn_t[:])
                    # output projection for batch b
                    for t in range(8):
                        ysb = ypool.tile([128, D], F32, tag="ysb")
                        for nb in range(4):
                            yps = py.tile([128, 512], F32, tag="y")
                            for h in range(NREP):
                                nc.tensor.matmul(
                                    yps[:],
                                    avn[b][h][:, bass.ts(t, 128)],
                                    wo_sb[:, h * D + nb * 512:h * D + (nb + 1) * 512],
                                    start=(h == 0), stop=(h == NREP - 1),
                                )
                            nc.vector.tensor_copy(ysb[:, bass.ts(nb, 512)], yps[:])
                        nc.sync.dma_start(out=y_d[b * S + t * 128:b * S + (t + 1) * 128, :], in_=ysb[:])

    try:
        nc.compile()
    finally:
        _bacc_mod.get_activation_tables = _orig_tables
    _CACHE[key] = nc
    return nc


def _host_prep(x, cos, sin, Wq, Wk, Wv, Wo):
    x = np.asarray(x, np.float32)
    xT = np.ascontiguousarray(x.reshape(T, D).T).astype(BF)
    cosT = np.asarray(cos, np.float32).T
    sinT = np.asarray(sin, np.float32).T
    cos2 = np.ascontiguousarray(np.tile(cosT, (2, 1)))          # (128, S) f32
    sin2 = np.ascontiguousarray(np.tile(sinT, (2, 1)))
    # lhsT for qshiftT = A @ qT  ->  arot = A.T (block-diag x2 over heads)
    A = np.zeros((HD, HD), np.float32)
    for d in range(32):
        A[d, d + 32] = -1.0
        A[32 + d, d] = 1.0
    arot = np.kron(np.eye(2, dtype=np.float32), A.T).astype(BF)  # (128,128)
    eye64 = np.eye(64, dtype=np.float32).astype(BF)

    Wq = np.asarray(Wq, np.float32)
    Wk = np.asarray(Wk, np.float32)
    Wv = np.asarray(Wv, np.float32)
    Wo = np.asarray(Wo, np.float32)
    in_maps = []
    for g in range(NC):
        wq_g = np.ascontiguousarray(Wq[:, g * QD:(g + 1) * QD]).astype(BF)
        wkv_g = np.ascontiguousarray(
            np.concatenate([Wk[:, g * HD:(g + 1) * HD], Wv[:, g * HD:(g + 1) * HD]], axis=1)
        ).astype(BF)
        wo_g = np.ascontiguousarray(Wo[g * QD:(g + 1) * QD, :]).astype(BF)
        in_maps.append({
            "xT": xT, "wq": wq_g, "wkv": wkv_g, "wo": wo_g,
            "cos2": cos2, "sin2": sin2, "arot": arot, "eye64": eye64,
        })
    return in_maps


def kernel(x, cos, sin, Wq, Wk, Wv, Wo):
    nc = _build()
    in_maps = _host_prep(x, cos, sin, Wq, Wk, Wv, Wo)
    res = bass_utils.run_bass_kernel_spmd(
        nc, in_maps, core_ids=list(range(NC)), trace=False,
    )
    y = np.zeros((T, D), np.float32)
    for r in res.results:
        y += np.asarray(r["y"], np.float32)
    return y.reshape(B, S, D)
